# revision 68
# baseline (speedup 1.0000x reference)
"""Causal self-attention (q=k=v) Trainium2 Bass kernel.

Problem: x [2, 2048, 1024] f32 -> causal MHA with 16 heads, head_dim 64,
q=k=v from x. Sharded head-parallel across 8 NeuronCores: 32 (batch, head)
pairs -> 4 heads per core, processed as 2 head-pairs.

Device algorithm per head (scores kept TRANSPOSED so the softmax k-reduction
is matmul-able):
  S^T[k, q] = K @ Q^T  via matmul(lhsT=xT[k-slice], rhs=xT[q-range])
  P^T = exp(S^T / 8)   (ACT, no max-subtraction: scores are O(1) for randn x)
  diag blocks masked by multiplying a precomputed upper-triangle 0/1 tile
  out'^T[65, q] = [V | 1].T @ P^T  accumulated over k-tiles in PSUM
    -> rows 0..63 = (P@V)^T, row 64 = softmax denominators
  transpose back per 128-q slice via PE, divide by denominator, DMA out.

All PE matmuls use float32r (rounded fp32, 1 cycle/row at N>=256, measured
~1.5e-4 rel err vs fp32's 4 cycles/row).
"""

import sys

if "/opt/trn_rl_repo" not in sys.path:
    sys.path.insert(0, "/opt/trn_rl_repo")

import numpy as np

B, S, D_MODEL, H, HD = 2, 2048, 1024, 16, 64
N_CORES = 8
PAIRS_PER_CORE = 2  # head-pairs of 128 columns each
NS = S // 128  # 16 s-chunks
NQ = S // 512  # 4 q-chunks of width 512
SCALE = 0.125  # 1/sqrt(64)

_CACHE = {}

# pool depths (tuned against the TimelineSim cost model)
PT_BUFS = 3  # exp-output (P^T) tiles
OT_BUFS = 3  # [65, 512] writeback staging tiles
OB_BUFS = 4  # coalesced output tiles
X_BUFS = 2  # per-pair input/V'/xT tiles (2 = prefetch next head-pair)


_ENG_SEM_PREFIX = {
    "EngineType.Activation": "Activation_",
    "EngineType.DVE": "DVE_",
    "EngineType.PE": "PE_",
    "EngineType.Pool": "Pool_",
    "EngineType.SP": "SP_",
}


def _strip_stale_self_waits(nc, min_slack=3):
    """Drop same-engine sem waits whose increment happened >= min_slack
    instructions earlier on that in-order engine: they are pool-slot WAW
    bookkeeping with thousands of cycles of real separation, but each one
    Tile emits costs ~58-97ns of engine queue time (often as a standalone
    hoisted EventSemaphore). Tight chains (slack < min_slack, e.g. the
    reciprocal->multiply RAW) are kept."""
    pos = {}
    sem_cum = {}
    inc_pos = {}
    n_stripped = 0
    for bb in nc.main_func.blocks:
        for inst in bb.instructions:
            eng = str(inst.engine)
            pref = _ENG_SEM_PREFIX.get(eng)
            p = pos.get(eng, 0)
            si = inst.sync_info
            if si is not None and si.on_wait and pref:
                keep = []
                for w in si.on_wait:
                    if w.ant_name.startswith(pref):
                        hist = inc_pos.get(w.ant_name)
                        if hist is not None and w.wait_value <= len(hist):
                            p_inc = hist[w.wait_value - 1]
                            if p - p_inc >= min_slack:
                                n_stripped += 1
                                continue
                    keep.append(w)
                if len(keep) != len(si.on_wait):
                    si.on_wait = keep
            if si is not None:
                for u in si.on_update:
                    nm = getattr(u, "ant_name", None)
                    if nm is None:
                        continue
                    if nm.startswith(pref or "\x00"):
                        sem_cum[nm] = sem_cum.get(nm, 0) + int(
                            getattr(u, "update_value", 1) or 1
                        )
                        inc_pos.setdefault(nm, [])
                        hist = inc_pos[nm]
                        while len(hist) < sem_cum[nm]:
                            hist.append(p)
            pos[eng] = p + 1
    return n_stripped


def _split_multi_waits(nc, mybir):
    """This container's walrus allows max 1 sync-wait per instruction; hoist
    extras into standalone EventSemaphore instructions on the same engine."""
    for bb in nc.main_func.blocks:
        out = []
        for inst in bb.instructions:
            si = inst.sync_info
            if si is not None and si.on_wait and len(si.on_wait) > 1:
                waits = list(si.on_wait)
                for i, wt in enumerate(waits[:-1]):
                    wi = mybir.InstEventSemaphore(
                        name=f"{inst.name}-sw{i}", ins=[], outs=[]
                    )
                    wi.engine = inst.engine
                    wi.sync_info = mybir.SyncInfo(on_wait=[wt], on_update=[])
                    out.append(wi)
                si.on_wait = [waits[-1]]
            out.append(inst)
        bb.instructions[:] = out


def _emit(ctx, tc, ys, xs):
    import concourse.bass as bass  # noqa: F401
    from concourse import mybir
    from concourse.masks import make_identity, make_upper_triangular

    nc = tc.nc
    f32 = mybir.dt.float32
    f32r = mybir.dt.float32r

    const = ctx.enter_context(tc.tile_pool(name="const", bufs=1))
    xpool = ctx.enter_context(tc.tile_pool(name="xpool", bufs=X_BUFS))
    vpool = ctx.enter_context(tc.tile_pool(name="vpool", bufs=X_BUFS))
    xtpool = ctx.enter_context(tc.tile_pool(name="xtpool", bufs=X_BUFS))
    ptpool = ctx.enter_context(tc.tile_pool(name="ptpool", bufs=PT_BUFS))
    otpool = ctx.enter_context(tc.tile_pool(name="otpool", bufs=OT_BUFS))
    obpool = ctx.enter_context(tc.tile_pool(name="obpool", bufs=OB_BUFS))
    recpool = ctx.enter_context(tc.tile_pool(name="recpool", bufs=8))
    ps_s = ctx.enter_context(tc.tile_pool(name="ps_s", bufs=2, space="PSUM"))
    ps_o = ctx.enter_context(tc.tile_pool(name="ps_o", bufs=2, space="PSUM"))
    ps_t = ctx.enter_context(tc.tile_pool(name="ps_t", bufs=1, space="PSUM"))
    ps_tb = ctx.enter_context(tc.tile_pool(name="ps_tb", bufs=1, space="PSUM"))

    ident = const.tile([128, 128], f32)
    make_identity(nc, ident)
    tri = const.tile([128, 128], f32)
    make_upper_triangular(nc, tri, val=1.0, diag=True)
    # [tri | tri] for the batched two-window diag mask (windows 640 apart)
    tri2 = const.tile([128, 2, 128], f32)
    nc.vector.tensor_copy(tri2[:, 0, :], tri)
    nc.vector.tensor_copy(tri2[:, 1, :], tri)
    # [zeros | tri] for the widened o=384 diag block (256-wide window)
    tri02 = const.tile([128, 256], f32)
    nc.vector.memset(tri02[:, 0:128], 0.0)
    nc.vector.tensor_copy(tri02[:, 128:256], tri)
    ones = const.tile([128, NS * 2], f32)
    nc.vector.memset(ones, 1.0)

    for p in range(PAIRS_PER_CORE):
        # load both heads' columns: [128 part, 16 s-chunks, 128 d]
        x2 = xpool.tile([128, NS, 128], f32)
        xs_r = xs[p].rearrange("(si sp) d -> sp si d", sp=128)
        for s0, s1 in ((0, 2), (2, 4), (4, 8), (8, NS)):
            nc.sync.dma_start(out=x2[:, s0:s1, :], in_=xs_r[:, s0:s1, :])
        # xT stacked pair: partitions 0-63 head A dims, 64-127 head B dims.
        # Built lazily per q-chunk inside the qc loop: qc only needs the
        # si <= 4qc+3 prefix, and the PE is in-order, so emitting all 16
        # transposes upfront would stall the first score matmuls behind the
        # whole input DMA.
        xt = xtpool.tile([128, S], f32r)
        # V' = [V | 1] per head, f32r-rounded: [128, si, head_in_pair, 65]
        vp = vpool.tile([128, NS, 2, 65], f32r)

        deferred = []

        def flush_writeback(limit=None):
            n = 0
            while deferred and (limit is None or n < limit):
                deferred.pop(0)()
                n += 1

        def build_slice(s0, s1):
            for si in range(s0, s1):
                tps = ps_t.tile([128, 128], f32, name="tps", tag="tp")
                nc.tensor.transpose(tps, x2[:, si, :], ident)
                nc.vector.tensor_copy(xt[:, si * 128 : (si + 1) * 128], tps)
            nc.vector.tensor_copy(vp[:, s0:s1, 0, 0:64], x2[:, s0:s1, 0:64])
            nc.vector.tensor_copy(vp[:, s0:s1, 1, 0:64], x2[:, s0:s1, 64:128])
            nc.vector.tensor_copy(
                vp[:, s0:s1, :, 64],
                ones[:, 2 * s0 : 2 * s1].rearrange("p (a b) -> p a b", b=2),
            )

        build_slice(0, NS)

        for qc in range(NQ):
            po = [
                ps_o.tile([65, 512], f32, name="po", tag="po") for _ in range(2)
            ]
            n_ki = 4 * qc + 4  # k-tiles 0..n_ki-1 (even, so pairs complete)
            # PV emission lags one chunk behind scores/exp: the first PV of a
            # qc waits on the po slot (freed by the previous qc's DVE ot
            # copy), and the in-order PE queue would stall all later score
            # matmuls behind that wait
            pending_pv = []
            for ck in range(n_ki // 2):
                prev_pv, pending_pv = pending_pv, []
                kis = [2 * ck, 2 * ck + 1]
                # raw diag offset; o=384 is widened to 256 so those matmuls
                # run N=256 at full f32r rate instead of N=128 at 1/4 rate
                raws = [128 * k - 512 * qc for k in kis]
                offs = [min(max(0, o), 256) for o in raws]
                width = 1024
                for h2 in range(2):
                    base = 64 * h2
                    pb = ps_s.tile([128, 1024], f32, name="pb", tag="pb")
                    for j, (k, o) in enumerate(zip(kis, offs)):
                        nc.tensor.matmul(
                            pb[:, j * 512 + o : (j + 1) * 512],
                            lhsT=xt[base : base + 64, 128 * k : 128 * (k + 1)],
                            rhs=xt[base : base + 64, 512 * qc + o : 512 * (qc + 1)],
                            start=True,
                            stop=True,
                        )
                    pt = ptpool.tile([128, 1024], f32r, name="pt", tag="pt")
                    nc.scalar.activation(
                        out=pt[:, offs[0] : width],
                        in_=pb[:, offs[0] : width],
                        func=mybir.ActivationFunctionType.Exp,
                        scale=SCALE,
                    )
                    if raws[0] == 0:
                        # diag pair (o=0, o=128): one batched two-window mask
                        # at cols [0:128) and [640:768), 640 apart
                        w0 = pt[:, 0:1]
                        win2 = bass.AP(
                            tensor=w0.tensor,
                            offset=w0.offset,
                            ap=[w0.ap[0], [640, 2], [1, 128]],
                        )
                        nc.vector.tensor_mul(win2, win2, tri2)
                    elif raws[0] == 256:
                        # diag pair (o=256, o=384): tri at [256:384) and
                        # [zeros|tri] over the widened [768:1024)
                        nc.vector.tensor_mul(pt[:, 256:384], pt[:, 256:384], tri)
                        nc.vector.tensor_mul(
                            pt[:, 768:1024], pt[:, 768:1024], tri02
                        )
                    def emit_pv(h2=h2, pt=pt, kis=kis, offs=offs):
                        for j, (k, o) in enumerate(zip(kis, offs)):
                            nc.tensor.matmul(
                                po[h2][:, o:512],
                                lhsT=vp[:, k, h2, :],
                                rhs=pt[:, j * 512 + o : (j + 1) * 512],
                                start=(k == 0),
                                stop=(k == n_ki - 1),
                            )

                    pending_pv.append(emit_pv)
                for fn in prev_pv:
                    fn()
            for fn in pending_pv:
                fn()
            # prompt PSUM->SBUF copies (free the po slots), then defer the
            # transpose/normalize/DMA tail one qc so next-qc scores keep the
            # ACT pipeline fed
            last = p == PAIRS_PER_CORE - 1 and qc == NQ - 1
            ots = []
            for h2 in range(2):
                ot = otpool.tile([65, 512], f32, name="ot", tag="ot")
                nc.vector.tensor_copy(ot, po[h2])
                ots.append(ot)

            def writeback(qc=qc, ots=ots, last=last):
                ob2 = obpool.tile([128, 4, 128], f32, name="ob2", tag="ob2")
                q0 = 512 * qc
                for h2 in range(2):
                    for t in range(4):
                        # the very last writeback has no scores running: use
                        # the idle 2-slot score pool so transposes pipeline
                        pool = ps_s if last else ps_tb
                        tb = pool.tile(
                            [128, 128], f32, name="tb", tag="pb" if last else "tb"
                        )
                        nc.tensor.transpose(
                            tb[:, 0:65],
                            ots[h2][:, t * 128 : (t + 1) * 128],
                            ident[0:65, 0:65],
                        )
                        rec = recpool.tile([128, 1], f32, name="rec", tag="rec")
                        nc.vector.reciprocal(rec, tb[:, 64:65])
                        nc.vector.tensor_scalar_mul(
                            ob2[:, t, 64 * h2 : 64 * (h2 + 1)], tb[:, 0:64], rec
                        )
                nc.gpsimd.dma_start(
                    out=ys[p, q0 : q0 + 512, :].rearrange(
                        "(t sp) d -> sp t d", sp=128
                    ),
                    in_=ob2,
                )

            flush_writeback()  # emit previous qc's deferred tail
            deferred.append(writeback)
        flush_writeback()


def _build_nc():
    import concourse.bass as bass
    import concourse.tile as tile
    from concourse import mybir

    nc = bass.Bass(
        "TRN2", target_bir_lowering=False, debug=False, num_devices=N_CORES
    )
    xs = nc.dram_tensor(
        "xs", [PAIRS_PER_CORE, S, 128], mybir.dt.float32, kind="ExternalInput"
    ).ap()
    ys = nc.dram_tensor(
        "ys", [PAIRS_PER_CORE, S, 128], mybir.dt.float32, kind="ExternalOutput"
    ).ap()
    from contextlib import ExitStack

    with tile.TileContext(nc) as tc:
        with ExitStack() as stack:
            _emit(stack, tc, ys, xs)
    _strip_stale_self_waits(nc)
    _split_multi_waits(nc, mybir)
    return nc


def _get_nc():
    if "nc" not in _CACHE:
        _CACHE["nc"] = _build_nc()
    return _CACHE["nc"]


def _shard_inputs(x):
    """x [B, S, D] -> per-core [PAIRS_PER_CORE, S, 128] head-pair blocks."""
    xh = np.ascontiguousarray(np.asarray(x, dtype=np.float32)).reshape(B, S, H, HD)
    in_maps = []
    for c in range(N_CORES):
        blocks = []
        for fp in (2 * c, 2 * c + 1):  # global pair index = b*8 + hp
            b, hp = divmod(fp, H // 2)
            blocks.append(xh[b, :, 2 * hp : 2 * hp + 2, :].reshape(S, 128))
        in_maps.append({"xs": np.ascontiguousarray(np.stack(blocks))})
    return in_maps


def _unshard_outputs(results):
    y = np.empty((B, S, H, HD), dtype=np.float32)
    for c in range(N_CORES):
        ys = results[c]["ys"]
        for j, fp in enumerate((2 * c, 2 * c + 1)):
            b, hp = divmod(fp, H // 2)
            y[b, :, 2 * hp : 2 * hp + 2, :] = ys[j].reshape(S, 2, HD)
    return y.reshape(B, S, D_MODEL)


def kernel(x, _collect=None):
    from concourse.bass_utils import run_bass_kernel_spmd

    nc = _get_nc()
    in_maps = _shard_inputs(x)
    kwargs = dict(_CACHE.get("run_kwargs", {}))
    res = run_bass_kernel_spmd(nc, in_maps, list(range(N_CORES)), **kwargs)
    if _collect is not None:
        _collect["res"] = res
    return _unshard_outputs(res.results)


# revision 69
# speedup vs baseline: 1.0159x; 1.0159x over previous
"""Causal self-attention (q=k=v) Trainium2 Bass kernel.

Problem: x [2, 2048, 1024] f32 -> causal MHA with 16 heads, head_dim 64,
q=k=v from x. Sharded head-parallel across 8 NeuronCores: 32 (batch, head)
pairs -> 4 heads per core, processed as 2 head-pairs.

Device algorithm per head (scores kept TRANSPOSED so the softmax k-reduction
is matmul-able):
  S^T[k, q] = K @ Q^T  via matmul(lhsT=xT[k-slice], rhs=xT[q-range])
  P^T = exp(S^T / 8)   (ACT, no max-subtraction: scores are O(1) for randn x)
  diag blocks masked by multiplying a precomputed upper-triangle 0/1 tile
  out'^T[65, q] = [V | 1].T @ P^T  accumulated over k-tiles in PSUM
    -> rows 0..63 = (P@V)^T, row 64 = softmax denominators
  transpose back per 128-q slice via PE, divide by denominator, DMA out.

All PE matmuls use float32r (rounded fp32, 1 cycle/row at N>=256, measured
~1.5e-4 rel err vs fp32's 4 cycles/row).
"""

import sys

if "/opt/trn_rl_repo" not in sys.path:
    sys.path.insert(0, "/opt/trn_rl_repo")

import numpy as np

B, S, D_MODEL, H, HD = 2, 2048, 1024, 16, 64
N_CORES = 8
PAIRS_PER_CORE = 2  # head-pairs of 128 columns each
NS = S // 128  # 16 s-chunks
NQ = S // 512  # 4 q-chunks of width 512
SCALE = 0.125  # 1/sqrt(64)

_CACHE = {}

# pool depths (tuned against the TimelineSim cost model)
PT_BUFS = 3  # exp-output (P^T) tiles
OT_BUFS = 3  # [65, 512] writeback staging tiles
OB_BUFS = 4  # coalesced output tiles
X_BUFS = 2  # per-pair input/V'/xT tiles (2 = prefetch next head-pair)


_ENG_SEM_PREFIX = {
    "EngineType.Activation": "Activation_",
    "EngineType.DVE": "DVE_",
    "EngineType.PE": "PE_",
    "EngineType.Pool": "Pool_",
    "EngineType.SP": "SP_",
}


def _strip_stale_self_waits(nc, min_slack=3):
    """Drop same-engine sem waits whose increment happened >= min_slack
    instructions earlier on that in-order engine: they are pool-slot WAW
    bookkeeping with thousands of cycles of real separation, but each one
    Tile emits costs ~58-97ns of engine queue time (often as a standalone
    hoisted EventSemaphore). Tight chains (slack < min_slack, e.g. the
    reciprocal->multiply RAW) are kept."""
    pos = {}
    sem_cum = {}
    inc_pos = {}
    n_stripped = 0
    for bb in nc.main_func.blocks:
        for inst in bb.instructions:
            eng = str(inst.engine)
            pref = _ENG_SEM_PREFIX.get(eng)
            p = pos.get(eng, 0)
            si = inst.sync_info
            if si is not None and si.on_wait and pref:
                keep = []
                for w in si.on_wait:
                    if w.ant_name.startswith(pref):
                        hist = inc_pos.get(w.ant_name)
                        if hist is not None and w.wait_value <= len(hist):
                            p_inc = hist[w.wait_value - 1]
                            if p - p_inc >= min_slack:
                                n_stripped += 1
                                continue
                    keep.append(w)
                if len(keep) != len(si.on_wait):
                    si.on_wait = keep
            if si is not None:
                for u in si.on_update:
                    nm = getattr(u, "ant_name", None)
                    if nm is None:
                        continue
                    if nm.startswith(pref or "\x00"):
                        sem_cum[nm] = sem_cum.get(nm, 0) + int(
                            getattr(u, "update_value", 1) or 1
                        )
                        inc_pos.setdefault(nm, [])
                        hist = inc_pos[nm]
                        while len(hist) < sem_cum[nm]:
                            hist.append(p)
            pos[eng] = p + 1
    return n_stripped


def _split_multi_waits(nc, mybir):
    """This container's walrus allows max 1 sync-wait per instruction; hoist
    extras into standalone EventSemaphore instructions on the same engine."""
    for bb in nc.main_func.blocks:
        out = []
        for inst in bb.instructions:
            si = inst.sync_info
            if si is not None and si.on_wait and len(si.on_wait) > 1:
                waits = list(si.on_wait)
                for i, wt in enumerate(waits[:-1]):
                    wi = mybir.InstEventSemaphore(
                        name=f"{inst.name}-sw{i}", ins=[], outs=[]
                    )
                    wi.engine = inst.engine
                    wi.sync_info = mybir.SyncInfo(on_wait=[wt], on_update=[])
                    out.append(wi)
                si.on_wait = [waits[-1]]
            out.append(inst)
        bb.instructions[:] = out


def _emit(ctx, tc, ys, xs):
    import concourse.bass as bass  # noqa: F401
    from concourse import mybir
    from concourse.masks import make_identity, make_upper_triangular

    nc = tc.nc
    f32 = mybir.dt.float32
    f32r = mybir.dt.float32r

    const = ctx.enter_context(tc.tile_pool(name="const", bufs=1))
    xpool = ctx.enter_context(tc.tile_pool(name="xpool", bufs=X_BUFS))
    vpool = ctx.enter_context(tc.tile_pool(name="vpool", bufs=X_BUFS))
    xtpool = ctx.enter_context(tc.tile_pool(name="xtpool", bufs=X_BUFS))
    ptpool = ctx.enter_context(tc.tile_pool(name="ptpool", bufs=PT_BUFS))
    otpool = ctx.enter_context(tc.tile_pool(name="otpool", bufs=OT_BUFS))
    obpool = ctx.enter_context(tc.tile_pool(name="obpool", bufs=OB_BUFS))
    recpool = ctx.enter_context(tc.tile_pool(name="recpool", bufs=8))
    ps_s = ctx.enter_context(tc.tile_pool(name="ps_s", bufs=2, space="PSUM"))
    ps_o = ctx.enter_context(tc.tile_pool(name="ps_o", bufs=2, space="PSUM"))
    ps_t = ctx.enter_context(tc.tile_pool(name="ps_t", bufs=1, space="PSUM"))
    ps_tb = ctx.enter_context(tc.tile_pool(name="ps_tb", bufs=1, space="PSUM"))

    ident = const.tile([128, 128], f32)
    make_identity(nc, ident)
    tri = const.tile([128, 128], f32)
    make_upper_triangular(nc, tri, val=1.0, diag=True)
    # [tri | tri] for the batched two-window diag mask (windows 640 apart)
    tri2 = const.tile([128, 2, 128], f32)
    nc.vector.tensor_copy(tri2[:, 0, :], tri)
    nc.vector.tensor_copy(tri2[:, 1, :], tri)
    # [zeros | tri] for the widened o=384 diag block (256-wide window)
    tri02 = const.tile([128, 256], f32)
    nc.vector.memset(tri02[:, 0:128], 0.0)
    nc.vector.tensor_copy(tri02[:, 128:256], tri)
    ones = const.tile([128, NS * 2], f32)
    nc.vector.memset(ones, 1.0)

    for p in range(PAIRS_PER_CORE):
        # load both heads' columns: [128 part, 16 s-chunks, 128 d]
        x2 = xpool.tile([128, NS, 128], f32)
        xs_r = xs[p].rearrange("(si sp) d -> sp si d", sp=128)
        for s0, s1 in ((0, 2), (2, 4), (4, 8), (8, NS)):
            nc.sync.dma_start(out=x2[:, s0:s1, :], in_=xs_r[:, s0:s1, :])
        # xT stacked pair: partitions 0-63 head A dims, 64-127 head B dims.
        # Built lazily per q-chunk inside the qc loop: qc only needs the
        # si <= 4qc+3 prefix, and the PE is in-order, so emitting all 16
        # transposes upfront would stall the first score matmuls behind the
        # whole input DMA.
        xt = xtpool.tile([128, S], f32r)
        # V' = [V | 1] per head, f32r-rounded: [128, si, head_in_pair, 65]
        vp = vpool.tile([128, NS, 2, 65], f32r)

        deferred = []

        def flush_writeback(limit=None):
            n = 0
            while deferred and (limit is None or n < limit):
                deferred.pop(0)()
                n += 1

        def build_slice(s0, s1):
            for si in range(s0, s1):
                # pair 0 runs before any writeback exists, so ps_tb is idle:
                # alternate pools to pipeline the startup transpose chain
                # that gates the very first score matmuls
                if p == 0 and si % 2 == 1:
                    tps = ps_tb.tile([128, 128], f32, name="tps", tag="tb")
                else:
                    tps = ps_t.tile([128, 128], f32, name="tps", tag="tp")
                nc.tensor.transpose(tps, x2[:, si, :], ident)
                nc.vector.tensor_copy(xt[:, si * 128 : (si + 1) * 128], tps)
            nc.vector.tensor_copy(vp[:, s0:s1, 0, 0:64], x2[:, s0:s1, 0:64])
            nc.vector.tensor_copy(vp[:, s0:s1, 1, 0:64], x2[:, s0:s1, 64:128])
            nc.vector.tensor_copy(
                vp[:, s0:s1, :, 64],
                ones[:, 2 * s0 : 2 * s1].rearrange("p (a b) -> p a b", b=2),
            )

        build_slice(0, NS)

        for qc in range(NQ):
            po = [
                ps_o.tile([65, 512], f32, name="po", tag="po") for _ in range(2)
            ]
            n_ki = 4 * qc + 4  # k-tiles 0..n_ki-1 (even, so pairs complete)
            # PV emission lags one chunk behind scores/exp: the first PV of a
            # qc waits on the po slot (freed by the previous qc's DVE ot
            # copy), and the in-order PE queue would stall all later score
            # matmuls behind that wait
            pending_pv = []
            for ck in range(n_ki // 2):
                prev_pv, pending_pv = pending_pv, []
                kis = [2 * ck, 2 * ck + 1]
                # raw diag offset; o=384 is widened to 256 so those matmuls
                # run N=256 at full f32r rate instead of N=128 at 1/4 rate
                raws = [128 * k - 512 * qc for k in kis]
                offs = [min(max(0, o), 256) for o in raws]
                width = 1024
                for h2 in range(2):
                    base = 64 * h2
                    pb = ps_s.tile([128, 1024], f32, name="pb", tag="pb")
                    for j, (k, o) in enumerate(zip(kis, offs)):
                        nc.tensor.matmul(
                            pb[:, j * 512 + o : (j + 1) * 512],
                            lhsT=xt[base : base + 64, 128 * k : 128 * (k + 1)],
                            rhs=xt[base : base + 64, 512 * qc + o : 512 * (qc + 1)],
                            start=True,
                            stop=True,
                        )
                    pt = ptpool.tile([128, 1024], f32r, name="pt", tag="pt")
                    nc.scalar.activation(
                        out=pt[:, offs[0] : width],
                        in_=pb[:, offs[0] : width],
                        func=mybir.ActivationFunctionType.Exp,
                        scale=SCALE,
                    )
                    if raws[0] == 0:
                        # diag pair (o=0, o=128): one batched two-window mask
                        # at cols [0:128) and [640:768), 640 apart
                        w0 = pt[:, 0:1]
                        win2 = bass.AP(
                            tensor=w0.tensor,
                            offset=w0.offset,
                            ap=[w0.ap[0], [640, 2], [1, 128]],
                        )
                        nc.vector.tensor_mul(win2, win2, tri2)
                    elif raws[0] == 256:
                        # diag pair (o=256, o=384): tri at [256:384) and
                        # [zeros|tri] over the widened [768:1024)
                        nc.vector.tensor_mul(pt[:, 256:384], pt[:, 256:384], tri)
                        nc.vector.tensor_mul(
                            pt[:, 768:1024], pt[:, 768:1024], tri02
                        )
                    def emit_pv(h2=h2, pt=pt, kis=kis, offs=offs):
                        for j, (k, o) in enumerate(zip(kis, offs)):
                            nc.tensor.matmul(
                                po[h2][:, o:512],
                                lhsT=vp[:, k, h2, :],
                                rhs=pt[:, j * 512 + o : (j + 1) * 512],
                                start=(k == 0),
                                stop=(k == n_ki - 1),
                            )

                    pending_pv.append(emit_pv)
                for fn in prev_pv:
                    fn()
            for fn in pending_pv:
                fn()
            # prompt PSUM->SBUF copies (free the po slots), then defer the
            # transpose/normalize/DMA tail one qc so next-qc scores keep the
            # ACT pipeline fed
            last = p == PAIRS_PER_CORE - 1 and qc == NQ - 1
            ots = []
            for h2 in range(2):
                ot = otpool.tile([65, 512], f32, name="ot", tag="ot")
                nc.vector.tensor_copy(ot, po[h2])
                ots.append(ot)

            def writeback(qc=qc, ots=ots, last=last):
                ob2 = obpool.tile([128, 4, 128], f32, name="ob2", tag="ob2")
                q0 = 512 * qc
                for h2 in range(2):
                    for t in range(4):
                        # the very last writeback has no scores running: use
                        # the idle 2-slot score pool so transposes pipeline
                        pool = ps_s if last else ps_tb
                        tb = pool.tile(
                            [128, 128], f32, name="tb", tag="pb" if last else "tb"
                        )
                        nc.tensor.transpose(
                            tb[:, 0:65],
                            ots[h2][:, t * 128 : (t + 1) * 128],
                            ident[0:65, 0:65],
                        )
                        rec = recpool.tile([128, 1], f32, name="rec", tag="rec")
                        nc.vector.reciprocal(rec, tb[:, 64:65])
                        nc.vector.tensor_scalar_mul(
                            ob2[:, t, 64 * h2 : 64 * (h2 + 1)], tb[:, 0:64], rec
                        )
                nc.gpsimd.dma_start(
                    out=ys[p, q0 : q0 + 512, :].rearrange(
                        "(t sp) d -> sp t d", sp=128
                    ),
                    in_=ob2,
                )

            flush_writeback()  # emit previous qc's deferred tail
            deferred.append(writeback)
        flush_writeback()


def _build_nc():
    import concourse.bass as bass
    import concourse.tile as tile
    from concourse import mybir

    nc = bass.Bass(
        "TRN2", target_bir_lowering=False, debug=False, num_devices=N_CORES
    )
    xs = nc.dram_tensor(
        "xs", [PAIRS_PER_CORE, S, 128], mybir.dt.float32, kind="ExternalInput"
    ).ap()
    ys = nc.dram_tensor(
        "ys", [PAIRS_PER_CORE, S, 128], mybir.dt.float32, kind="ExternalOutput"
    ).ap()
    from contextlib import ExitStack

    with tile.TileContext(nc) as tc:
        with ExitStack() as stack:
            _emit(stack, tc, ys, xs)
    _strip_stale_self_waits(nc)
    _split_multi_waits(nc, mybir)
    return nc


def _get_nc():
    if "nc" not in _CACHE:
        _CACHE["nc"] = _build_nc()
    return _CACHE["nc"]


def _shard_inputs(x):
    """x [B, S, D] -> per-core [PAIRS_PER_CORE, S, 128] head-pair blocks."""
    xh = np.ascontiguousarray(np.asarray(x, dtype=np.float32)).reshape(B, S, H, HD)
    in_maps = []
    for c in range(N_CORES):
        blocks = []
        for fp in (2 * c, 2 * c + 1):  # global pair index = b*8 + hp
            b, hp = divmod(fp, H // 2)
            blocks.append(xh[b, :, 2 * hp : 2 * hp + 2, :].reshape(S, 128))
        in_maps.append({"xs": np.ascontiguousarray(np.stack(blocks))})
    return in_maps


def _unshard_outputs(results):
    y = np.empty((B, S, H, HD), dtype=np.float32)
    for c in range(N_CORES):
        ys = results[c]["ys"]
        for j, fp in enumerate((2 * c, 2 * c + 1)):
            b, hp = divmod(fp, H // 2)
            y[b, :, 2 * hp : 2 * hp + 2, :] = ys[j].reshape(S, 2, HD)
    return y.reshape(B, S, D_MODEL)


def kernel(x, _collect=None):
    from concourse.bass_utils import run_bass_kernel_spmd

    nc = _get_nc()
    in_maps = _shard_inputs(x)
    kwargs = dict(_CACHE.get("run_kwargs", {}))
    res = run_bass_kernel_spmd(nc, in_maps, list(range(N_CORES)), **kwargs)
    if _collect is not None:
        _collect["res"] = res
    return _unshard_outputs(res.results)
